# revision 2
# baseline (speedup 1.0000x reference)
"""MLA (Multi-Head Latent Attention) Bass kernel for 8 Trainium2 NeuronCores.

Sharding: 8 cores = 2 (batch) x 4 (head groups). Core c -> batch c//4,
group g=c%4 owning heads {2g, 2g+1, 2g+8, 2g+9} (paired h/h+8 so the
rotate-half RoPE over d_model=2048 stays core-local).

All activations flow on-device in transposed [feature, token] layout so no
on-chip transposes are needed (the host pre-transposes x). Attention scores
are computed in [k, q] layout; the softmax denominator is computed with an
all-ones matmul on the PE (scores are bounded, so no max subtraction), exp
runs on the scalar engine straight out of PSUM, and 1/denom is folded into
the attention-output scaling. Matmuls use fp32r (full PE rate at N=512).

Each core computes a partial out^T = (attn_out_g @ Wout[rows_g]).T for its
4 heads; the host sums the 4 partials per batch and transposes. bout is
added on-device by the g==0 cores only.
"""
import os
import sys

if "/opt/trn_rl_repo" not in sys.path:
    sys.path.insert(0, "/opt/trn_rl_repo")

import numpy as np

D_MODEL = 2048
Q_LAT = 1536
KV_LAT = 512
NUM_HEADS = 16
HD = 128
B, S = 2, 2048
SCALE = 1.0 / np.sqrt(2.0 * HD)  # 1/16

QT = 512          # query tile width (matmul free dim)
NQT = S // QT     # 4
NC_DM = D_MODEL // 128   # 16 chunks of the model dim
NC_QL = Q_LAT // 128     # 12
NC_KV = KV_LAT // 128    # 4
NKC = S // 128           # 16 key chunks

_CACHE = {}
LAST_RESULT = None


def _strip_cols(g):
    """Global column ranges (width 128) of the 4 local head strips, in local
    order [2g, 2g+1, 2g+8, 2g+9]."""
    return [256 * g, 256 * g + 128, 1024 + 256 * g, 1024 + 256 * g + 128]


def _build_bass():
    import concourse.bass as bass
    from concourse import bacc, mybir
    from concourse.tile import TileContext

    f32 = mybir.dt.float32
    f32r = mybir.dt.float32r
    AF = mybir.ActivationFunctionType

    nc = bacc.Bacc("TRN2", target_bir_lowering=False, debug=False)

    def inp(name, shape, dt=None):
        return nc.dram_tensor(name, list(shape), dt or f32r, kind="ExternalInput")

    xqT = inp("xqT", (D_MODEL, S))
    xkT = inp("xkT", (D_MODEL, S))
    wq_down = inp("wq_down", (NC_QL, 128, NC_DM * 128))     # [s][p][c*128+f]
    wkv_down = inp("wkv_down", (NC_KV, 128, NC_DM * 128))
    wk_rope = inp("wk_rope", (128, NC_DM * 128))            # [p][c*128+f]
    wq_up = inp("wq_up", (4, 128, NC_QL * 128))             # [strip][p][c*128+f]
    wq_rope = inp("wq_rope", (4, 128, NC_QL * 128))
    wk_up = inp("wk_up", (4, 128, NC_KV * 128))
    wv_up = inp("wv_up", (128, NC_KV * 512))                # [p][c*512+f]
    wout = inp("wout", (128, 64 * 128))                     # [p][(m*4+h)*128+f]
    cos_q = inp("cos_q", (2, 128, S), f32)                       # [block j][d][q]
    sin_q = inp("sin_q", (2, 128, S), f32)
    cos_k = inp("cos_k", (64, S), f32)
    sin_k = inp("sin_k", (64, S), f32)
    masks = inp("masks", (128, 4 * QT))                     # [kl][(o*QT)+ql]
    ones = inp("ones", (128, 128))
    bias = inp("bias", (128, NC_DM), f32)                        # [p][m]

    outT = nc.dram_tensor("outT", [D_MODEL, S], f32, kind="ExternalOutput")

    # DRAM scratch for inter-phase spills
    latq_d = nc.dram_tensor("latq_d", [NC_QL, 128, S], f32r, kind="Internal")
    qnew_d = nc.dram_tensor("qnew_d", [8, 128, S], f32r, kind="Internal")
    kproj_d = nc.dram_tensor("kproj_d", [4, 128, S], f32r, kind="Internal")
    krope_d = nc.dram_tensor("krope_d", [128, S], f32r, kind="Internal")
    v_d = nc.dram_tensor("v_d", [NKC, 128, 512], f32r, kind="Internal")

    def r(ap):
        return ap

    xqT_v = xqT.ap().rearrange("(c p) q -> p c q", p=128)   # [128, 16, 2048]
    xkT_v = xkT.ap().rearrange("(c p) q -> p c q", p=128)

    with TileContext(nc) as tc:
        # ---------------- Phase 2: latqT = (xq @ Wq_down)^T -------------
        with tc.tile_pool(name="p2w", bufs=1) as p2w, \
             tc.tile_pool(name="p2x", bufs=2) as p2x, \
             tc.tile_pool(name="p2c", bufs=3) as p2c, \
             tc.tile_pool(name="p2ps", bufs=2, space="PSUM") as p2ps:
            wqd_sb = p2w.tile([128, NC_QL * NC_DM * 128], f32r)  # 96KB/part
            for s in range(NC_QL):
                nc.sync.dma_start(
                    out=wqd_sb[:, s * NC_DM * 128:(s + 1) * NC_DM * 128],
                    in_=wq_down.ap()[s])
            for qt in range(NQT):
                xq_t = p2x.tile([128, NC_DM, QT], f32r, tag="xq")
                nc.sync.dma_start(out=xq_t, in_=xqT_v[:, :, qt * QT:(qt + 1) * QT])
                for s in range(NC_QL):
                    ps = p2ps.tile([128, QT], f32, tag="ps")
                    for c in range(NC_DM):
                        nc.tensor.matmul(
                            ps, r(wqd_sb[:, (s * NC_DM + c) * 128:(s * NC_DM + c + 1) * 128]),
                            r(xq_t[:, c, :]), start=(c == 0), stop=(c == NC_DM - 1))
                    cp = p2c.tile([128, QT], f32r, tag="cp")
                    nc.scalar.copy(out=cp, in_=ps)
                    nc.sync.dma_start(
                        out=latq_d.ap()[s][:, qt * QT:(qt + 1) * QT], in_=cp)

        # ------------- Phase 3: q_projT + ropeT per head strip ----------
        with tc.tile_pool(name="p3w", bufs=1) as p3w, \
             tc.tile_pool(name="p3l", bufs=2) as p3l, \
             tc.tile_pool(name="p3t", bufs=2) as p3t, \
             tc.tile_pool(name="p3ps", bufs=2, space="PSUM") as p3ps:
            wqu_sb = p3w.tile([128, 4 * NC_QL * 128], f32r)
            wqr_sb = p3w.tile([128, 4 * NC_QL * 128], f32r)
            cosq_sb = p3w.tile([128, 2, S], f32)
            sinq_sb = p3w.tile([128, 2, S], f32)
            for s in range(4):
                nc.sync.dma_start(
                    out=wqu_sb[:, s * NC_QL * 128:(s + 1) * NC_QL * 128],
                    in_=wq_up.ap()[s])
                nc.sync.dma_start(
                    out=wqr_sb[:, s * NC_QL * 128:(s + 1) * NC_QL * 128],
                    in_=wq_rope.ap()[s])
            for j in range(2):
                nc.sync.dma_start(out=cosq_sb[:, j, :], in_=cos_q.ap()[j])
                nc.sync.dma_start(out=sinq_sb[:, j, :], in_=sin_q.ap()[j])

            for qt in range(NQT):
                q0 = qt * QT
                lat_t = p3l.tile([128, NC_QL, QT], f32r, tag="lat")
                nc.sync.dma_start(
                    out=lat_t,
                    in_=latq_d.ap().rearrange("s p q -> p s q")[:, :, q0:q0 + QT])
                raw = []
                for s in range(4):
                    # q_proj strip
                    ps = p3ps.tile([128, QT], f32, tag="ps")
                    for c in range(NC_QL):
                        nc.tensor.matmul(
                            ps, r(wqu_sb[:, (s * NC_QL + c) * 128:(s * NC_QL + c + 1) * 128]),
                            r(lat_t[:, c, :]), start=(c == 0), stop=(c == NC_QL - 1))
                    cp = p3t.tile([128, QT], f32r, tag=f"qp{s}")
                    nc.scalar.copy(out=cp, in_=ps)
                    nc.sync.dma_start(out=qnew_d.ap()[2 * s][:, q0:q0 + QT], in_=cp)
                    # q_rope raw strip
                    ps2 = p3ps.tile([128, QT], f32, tag="ps")
                    for c in range(NC_QL):
                        nc.tensor.matmul(
                            ps2, r(wqr_sb[:, (s * NC_QL + c) * 128:(s * NC_QL + c + 1) * 128]),
                            r(lat_t[:, c, :]), start=(c == 0), stop=(c == NC_QL - 1))
                    rw = p3t.tile([128, QT], f32, tag=f"raw{s}")
                    nc.scalar.copy(out=rw, in_=ps2)
                    raw.append(rw)
                # rope combine: strips 0,1 = block A (j=0,1); 2,3 = block B
                for j in range(2):
                    a, b = raw[j], raw[2 + j]
                    cj = cosq_sb[:, j, q0:q0 + QT]
                    sj = sinq_sb[:, j, q0:q0 + QT]
                    t1 = p3t.tile([128, QT], f32, tag=f"t1{j}")
                    t2 = p3t.tile([128, QT], f32, tag=f"t2{j}")
                    outa = p3t.tile([128, QT], f32r, tag=f"oa{j}")
                    outb = p3t.tile([128, QT], f32r, tag=f"ob{j}")
                    nc.vector.tensor_mul(t1, a, cj)
                    nc.vector.tensor_mul(t2, b, sj)
                    nc.vector.tensor_sub(outa, t1, t2)
                    nc.sync.dma_start(out=qnew_d.ap()[2 * j + 1][:, q0:q0 + QT], in_=outa)
                    nc.vector.tensor_mul(t1, b, cj)
                    nc.vector.tensor_mul(t2, a, sj)
                    nc.vector.tensor_add(outb, t1, t2)
                    nc.sync.dma_start(out=qnew_d.ap()[2 * (2 + j) + 1][:, q0:q0 + QT], in_=outb)

        # ------------- Phase 1: K/V build (latkv, k_proj, V, k_rope) ----
        with tc.tile_pool(name="p1w", bufs=1) as p1w, \
             tc.tile_pool(name="p1x", bufs=2) as p1x, \
             tc.tile_pool(name="p1l", bufs=1) as p1l, \
             tc.tile_pool(name="p1t", bufs=2) as p1t, \
             tc.tile_pool(name="p1ps", bufs=2, space="PSUM") as p1ps:
            wkv_sb = p1w.tile([128, NC_KV * NC_DM * 128], f32r)
            for s in range(NC_KV):
                nc.sync.dma_start(
                    out=wkv_sb[:, s * NC_DM * 128:(s + 1) * NC_DM * 128],
                    in_=wkv_down.ap()[s])
            wkr_sb = p1w.tile([128, NC_DM * 128], f32r)
            nc.sync.dma_start(out=wkr_sb, in_=wk_rope.ap())
            wku_sb = p1w.tile([128, 4 * NC_KV * 128], f32r)
            for s in range(4):
                nc.sync.dma_start(
                    out=wku_sb[:, s * NC_KV * 128:(s + 1) * NC_KV * 128],
                    in_=wk_up.ap()[s])
            wvu_sb = p1w.tile([128, NC_KV * 512], f32r)
            nc.sync.dma_start(out=wvu_sb, in_=wv_up.ap())
            cosk_sb = p1w.tile([64, S], f32)
            sink_sb = p1w.tile([64, S], f32)
            nc.sync.dma_start(out=cosk_sb, in_=cos_k.ap())
            nc.sync.dma_start(out=sink_sb, in_=sin_k.ap())

            for kh in range(2):  # k halves of 1024
                k0 = kh * 1024
                latkv = p1l.tile([128, NC_KV, 1024], f32r, tag="latkv")
                krraw = p1l.tile([128, 1024], f32, tag="krraw")
                for kt in range(2):  # two 512-tiles within the half
                    kk = k0 + kt * QT
                    xk_t = p1x.tile([128, NC_DM, QT], f32r, tag="xk")
                    nc.sync.dma_start(out=xk_t, in_=xkT_v[:, :, kk:kk + QT])
                    for s in range(NC_KV):
                        ps = p1ps.tile([128, QT], f32, tag="ps")
                        for c in range(NC_DM):
                            nc.tensor.matmul(
                                ps, r(wkv_sb[:, (s * NC_DM + c) * 128:(s * NC_DM + c + 1) * 128]),
                                r(xk_t[:, c, :]), start=(c == 0), stop=(c == NC_DM - 1))
                        nc.scalar.copy(out=latkv[:, s, kt * QT:(kt + 1) * QT], in_=ps)
                    ps = p1ps.tile([128, QT], f32, tag="ps")
                    for c in range(NC_DM):
                        nc.tensor.matmul(
                            ps, r(wkr_sb[:, c * 128:(c + 1) * 128]),
                            r(xk_t[:, c, :]), start=(c == 0), stop=(c == NC_DM - 1))
                    nc.scalar.copy(out=krraw[:, kt * QT:(kt + 1) * QT], in_=ps)

                # k_rope combine for this half
                krb = p1t.tile([64, 1024], f32, tag="krb")
                nc.sync.dma_start(out=krb, in_=krraw[64:128, :])
                ck = cosk_sb[:, k0:k0 + 1024]
                sk = sink_sb[:, k0:k0 + 1024]
                t1 = p1t.tile([64, 1024], f32, tag="krt1")
                t2 = p1t.tile([64, 1024], f32, tag="krt2")
                otop = p1t.tile([64, 1024], f32r, tag="krot")
                obot = p1t.tile([64, 1024], f32r, tag="krob")
                nc.vector.tensor_mul(t1, krraw[0:64, :], ck)
                nc.vector.tensor_mul(t2, krb, sk)
                nc.vector.tensor_sub(otop, t1, t2)
                nc.sync.dma_start(out=krope_d.ap()[0:64, k0:k0 + 1024], in_=otop)
                nc.vector.tensor_mul(t1, krb, ck)
                nc.vector.tensor_mul(t2, krraw[0:64, :], sk)
                nc.vector.tensor_add(obot, t1, t2)
                nc.sync.dma_start(out=krope_d.ap()[64:128, k0:k0 + 1024], in_=obot)

                # k_projT strips for this half
                for s in range(4):
                    for kt in range(2):
                        ps = p1ps.tile([128, QT], f32, tag="ps")
                        for c in range(NC_KV):
                            nc.tensor.matmul(
                                ps, r(wku_sb[:, (s * NC_KV + c) * 128:(s * NC_KV + c + 1) * 128]),
                                r(latkv[:, c, kt * QT:(kt + 1) * QT]),
                                start=(c == 0), stop=(c == NC_KV - 1))
                        cp = p1t.tile([128, QT], f32r, tag="kpcp")
                        nc.scalar.copy(out=cp, in_=ps)
                        nc.sync.dma_start(
                            out=kproj_d.ap()[s][:, k0 + kt * QT:k0 + (kt + 1) * QT],
                            in_=cp)
                # V natural for this half
                for kc in range(8):  # 128-chunks within the half
                    ps = p1ps.tile([128, 512], f32, tag="ps")
                    for c in range(NC_KV):
                        nc.tensor.matmul(
                            ps, r(latkv[:, c, kc * 128:(kc + 1) * 128]),
                            r(wvu_sb[:, c * 512:(c + 1) * 512]),
                            start=(c == 0), stop=(c == NC_KV - 1))
                    cp = p1t.tile([128, 512], f32r, tag="vcp")
                    nc.scalar.copy(out=cp, in_=ps)
                    nc.sync.dma_start(out=v_d.ap()[kh * 8 + kc], in_=cp)

        # ------------- Phase 4: attention + output projection -----------
        with tc.tile_pool(name="p4kv", bufs=1) as p4kv, \
             tc.tile_pool(name="p4w", bufs=1) as p4w, \
             tc.tile_pool(name="p4q", bufs=2) as p4q, \
             tc.tile_pool(name="p4e", bufs=4) as p4e, \
             tc.tile_pool(name="p4a", bufs=2) as p4a, \
             tc.tile_pool(name="p4o", bufs=2) as p4o, \
             tc.tile_pool(name="p4ps", bufs=2, space="PSUM") as p4ps, \
             tc.tile_pool(name="p4pd", bufs=2, space="PSUM") as p4pd, \
             tc.tile_pool(name="p4pv", bufs=2, space="PSUM") as p4pv, \
             tc.tile_pool(name="p4po", bufs=2, space="PSUM") as p4po:
            kproj_sb = p4kv.tile([128, 4, S], f32r)
            nc.sync.dma_start(out=kproj_sb, in_=kproj_d.ap().rearrange("s p k -> p s k"))
            krope_sb = p4kv.tile([128, S], f32r)
            nc.sync.dma_start(out=krope_sb, in_=krope_d.ap())
            v_sb = p4kv.tile([128, NKC, 512], f32r)
            nc.sync.dma_start(out=v_sb, in_=v_d.ap().rearrange("c p f -> p c f"))
            wout_sb = p4w.tile([128, 64 * 128], f32r)
            nc.sync.dma_start(out=wout_sb, in_=wout.ap())
            masks_sb = p4w.tile([128, 4 * QT], f32r)
            nc.sync.dma_start(out=masks_sb, in_=masks.ap())
            ones_sb = p4w.tile([128, 128], f32r)
            nc.sync.dma_start(out=ones_sb, in_=ones.ap())
            bias_sb = p4w.tile([128, NC_DM], f32)
            nc.sync.dma_start(out=bias_sb, in_=bias.ap())

            for qt in range(NQT):
                q0 = qt * QT
                K = (q0 + QT) // 128  # causal: chunks 0..K-1
                qn = p4q.tile([128, 8, QT], f32r, tag="qn")
                nc.sync.dma_start(
                    out=qn, in_=qnew_d.ap().rearrange("s p q -> p s q")[:, :, q0:q0 + QT])
                attn = p4a.tile([128, 4, QT], f32r, tag="attn")
                for h in range(4):
                    psd = p4pd.tile([128, QT], f32, tag="psd")
                    psv = p4pv.tile([128, QT], f32, tag="psv")
                    for kc in range(K):
                        pss = p4ps.tile([128, QT], f32, tag="pss")
                        nc.tensor.matmul(
                            pss, r(kproj_sb[:, h, kc * 128:(kc + 1) * 128]),
                            r(qn[:, 2 * h, :]), start=True, stop=False)
                        nc.tensor.matmul(
                            pss, r(krope_sb[:, kc * 128:(kc + 1) * 128]),
                            r(qn[:, 2 * h + 1, :]), start=False, stop=True)
                        ex = p4e.tile([128, QT], f32r, tag="ex")
                        nc.scalar.activation(out=ex, in_=pss, func=AF.Exp, scale=float(SCALE))
                        o = kc - q0 // 128
                        if o >= 0:  # diagonal chunk: apply causal mask
                            nc.vector.tensor_mul(ex, ex, masks_sb[:, o * QT:(o + 1) * QT])
                        nc.tensor.matmul(
                            psd, r(ones_sb), r(ex),
                            start=(kc == 0), stop=(kc == K - 1), skip_group_check=True)
                        nc.tensor.matmul(
                            psv, r(v_sb[:, kc, h * 128:(h + 1) * 128]), r(ex),
                            start=(kc == 0), stop=(kc == K - 1), skip_group_check=True)
                    rec = p4e.tile([128, QT], f32, tag="rec")
                    nc.vector.reciprocal(rec, psd)
                    nc.vector.tensor_mul(attn[:, h, :], psv, rec)
                # output projection for this q tile
                for m in range(NC_DM):
                    pso = p4po.tile([128, QT], f32, tag="pso")
                    for h in range(4):
                        nc.tensor.matmul(
                            pso, r(wout_sb[:, (m * 4 + h) * 128:(m * 4 + h + 1) * 128]),
                            r(attn[:, h, :]), start=(h == 0), stop=(h == 3))
                    oc = p4o.tile([128, QT], f32, tag="oc")
                    nc.scalar.activation(
                        out=oc, in_=pso, func=AF.Identity,
                        bias=bias_sb[:, m:m + 1], scale=1.0)
                    nc.sync.dma_start(
                        out=outT.ap()[m * 128:(m + 1) * 128, q0:q0 + QT], in_=oc)

    nc.finalize()
    return nc


def _host_pack(inputs):
    """Build the 8 per-core input maps from the full inputs."""
    xq = np.ascontiguousarray(inputs["inputs_q"], dtype=np.float32)
    xk = np.ascontiguousarray(inputs["inputs_k"], dtype=np.float32)
    Wq_down = np.asarray(inputs["Wq_down"], dtype=np.float32)
    Wkv_down = np.asarray(inputs["Wkv_down"], dtype=np.float32)
    Wq_up = np.asarray(inputs["Wq_up"], dtype=np.float32)
    Wk_up = np.asarray(inputs["Wk_up"], dtype=np.float32)
    Wv_up = np.asarray(inputs["Wv_up"], dtype=np.float32)
    Wq_rope = np.asarray(inputs["Wq_rope"], dtype=np.float32)
    Wk_rope = np.asarray(inputs["Wk_rope"], dtype=np.float32)
    Wout = np.asarray(inputs["Wout"], dtype=np.float32)
    bout = np.asarray(inputs["bout"], dtype=np.float32)

    def pack_lhs(W, n_strips, strip_starts, nchunks):
        # -> [n_strips, 128, nchunks*128]: [s][p][c*128+f]
        out = np.empty((n_strips, 128, nchunks * 128), dtype=np.float32)
        for s in range(n_strips):
            blk = W[:, strip_starts[s]:strip_starts[s] + 128]  # [nchunks*128, 128]
            out[s] = blk.reshape(nchunks, 128, 128).transpose(1, 0, 2).reshape(128, -1)
        return out

    xqT = [np.ascontiguousarray(xq[b].T) for b in range(B)]
    xkT = [np.ascontiguousarray(xk[b].T) for b in range(B)]

    wq_down_p = pack_lhs(Wq_down, NC_QL, [128 * s for s in range(NC_QL)], NC_DM)
    wkv_down_p = pack_lhs(Wkv_down, NC_KV, [128 * s for s in range(NC_KV)], NC_DM)
    wk_rope_p = pack_lhs(Wk_rope, 1, [0], NC_DM)[0]

    # rope tables
    iq = np.arange(1024, dtype=np.float64)
    inv_q = 1.0 / (10000.0 ** (iq * 2.0 / D_MODEL))
    pos = np.arange(S, dtype=np.float64)
    ang_q = pos[:, None] * inv_q[None, :]          # [S, 1024]
    ik = np.arange(64, dtype=np.float64)
    inv_k = 1.0 / (10000.0 ** (ik * 2.0 / HD))
    ang_k = pos[:, None] * inv_k[None, :]          # [S, 64]
    cos_k = np.ascontiguousarray(np.cos(ang_k).T.astype(np.float32))  # [64, S]
    sin_k = np.ascontiguousarray(np.sin(ang_k).T.astype(np.float32))

    # causal diag masks [128, 4*QT]
    kl = np.arange(128)[:, None]
    ql = np.arange(QT)[None, :]
    masks = np.concatenate(
        [(kl + 128 * o <= ql).astype(np.float32) for o in range(4)], axis=1)
    masks = np.ascontiguousarray(masks)
    ones = np.ones((128, 128), dtype=np.float32)

    in_maps = []
    for c in range(8):
        b, g = divmod(c, 4)
        cols = _strip_cols(g)
        wq_up_p = pack_lhs(Wq_up, 4, cols, NC_QL)
        wq_rope_p = pack_lhs(Wq_rope, 4, cols, NC_QL)
        wk_up_p = pack_lhs(Wk_up, 4, cols, NC_KV)
        # wv_up: [128, nc_kv*512]; cols4 concatenated in local order
        cols4 = np.concatenate([np.arange(cs, cs + 128) for cs in cols])
        Wv_g = Wv_up[:, cols4]                      # [512, 512]
        wv_up_p = Wv_g.reshape(NC_KV, 128, 512).transpose(1, 0, 2).reshape(128, -1)
        # wout: rows for local heads; [128, 64*128] = [p][(m*4+h)*128+f]
        Wout_g = Wout[cols4, :].reshape(4, 128, NC_DM, 128)   # [h][p][m][f]
        wout_p = np.ascontiguousarray(
            Wout_g.transpose(1, 2, 0, 3).reshape(128, -1))    # [p][m,h,f]
        # cos/sin q for blocks j=0,1: global cols 256g+128j+d (<1024)
        cos_q_p = np.empty((2, 128, S), dtype=np.float32)
        sin_q_p = np.empty((2, 128, S), dtype=np.float32)
        for j in range(2):
            idx = 256 * g + 128 * j + np.arange(128)
            cos_q_p[j] = np.cos(ang_q[:, idx]).T
            sin_q_p[j] = np.sin(ang_q[:, idx]).T
        bias_p = (bout if g == 0 else np.zeros_like(bout)).reshape(NC_DM, 128)
        bias_p = np.ascontiguousarray(bias_p.T)     # [128, m]

        in_maps.append({
            "xqT": xqT[b], "xkT": xkT[b],
            "wq_down": wq_down_p, "wkv_down": wkv_down_p, "wk_rope": wk_rope_p,
            "wq_up": wq_up_p, "wq_rope": wq_rope_p, "wk_up": wk_up_p,
            "wv_up": np.ascontiguousarray(wv_up_p), "wout": wout_p,
            "cos_q": cos_q_p, "sin_q": sin_q_p, "cos_k": cos_k, "sin_k": sin_k,
            "masks": masks, "ones": ones, "bias": bias_p,
        })
    return in_maps


def kernel(**inputs):
    global LAST_RESULT
    from concourse.bass_utils import run_bass_kernel_spmd

    if "nc" not in _CACHE:
        _CACHE["nc"] = _build_bass()
    nc = _CACHE["nc"]

    in_maps = _host_pack(inputs)
    kwargs = {}
    if os.environ.get("KERNEL_TRACE"):
        try:
            sys.path.insert(0, os.path.dirname(os.path.abspath(__file__)))
            import axon_shim
            axon_shim.install()
        except Exception:
            pass
        kwargs["trace"] = True
    res = run_bass_kernel_spmd(nc, in_maps, core_ids=list(range(8)), **kwargs)
    LAST_RESULT = res

    out = np.empty((B, S, D_MODEL), dtype=np.float32)
    for b in range(B):
        acc = res.results[4 * b]["outT"].copy()
        for g in range(1, 4):
            acc += res.results[4 * b + g]["outT"]
        out[b] = acc.T
    return out
